# revision 8
# baseline (speedup 1.0000x reference)
"""KAN-SE (squeeze-excite with 2-layer KAN MLP) Trainium2 kernel.

Full-input contract: kernel(**inputs) takes the complete (32, 512, 64, 64)
batch plus KAN weights, shards the batch across 8 NeuronCores (4 samples
per core, data-parallel, weights replicated), and returns the full output.

Per-core device program (pure SPMD, no collectives), built so the load
and store streams overlap fully and share the ~358 GB/s per-core HBM
bandwidth:

  - ALL 16 x-tiles (4 samples x 4 channel-groups of (128, 4096) f32) are
    loaded up front on the SP HWDGE ring into a 3-deep f32 staging pool;
    the load DMAs are gated only by staging-slot recycling, never by the
    gate computation.
  - Each staged tile is converted f32->bf16 into the resident pool by a
    ScalarE Copy activation whose accum_out simultaneously produces the
    per-channel raw sum (the reduce rides the conversion). bf16 residency
    halves SBUF so all 4 samples stay on-chip.
  - Per-sample 2-layer KAN on the sums (mean normalization folded into
    the layer-1 knot tables / base weights; B-spline bases via
    Cox-de-Boor on VectorE; einsums as small PE matmuls accumulating in
    PSUM; all activations via Sigmoid only - silu(x) = x*sigmoid(x) with
    a DVE multiply - so the ScalarE table is loaded once).
  - Gate scales applied in-place on the bf16 tiles (VectorE).
  - Stores are bf16->f32 casting DMAs on the SWDGE (Pool) ring - the only
    traffic on that ring - so they drain in gate order concurrently with
    the remaining loads on the SP ring.
  - Weight/table constants load on the ACT HWDGE ring, off the load path.

HBM traffic is the 2x minimum (8 MiB in + 8 MiB out per sample per core);
the bf16 cast costs ~0.2% relative error on the output, well inside the
tolerance.
"""

import numpy as np

# ---- problem constants (hardcoded per contract; do not read spec/reference) ----
B, C, H, W = 32, 512, 64, 64
HIDDEN = 64            # max(16, 512 // 8)
KB = 8                 # GRID_SIZE + SPLINE_ORDER = 5 + 3
NCORES = 8
NS = B // NCORES       # samples per core = 4
NG = C // 128          # channel groups of 128 = 4
HWPIX = H * W          # 4096

# gtab column layout: [G0(12) | -g_i for k=1(10) | g_{i+2} k=1(10)
#                      | -g_i k=2(9) | g_{i+3} k=2(9) | -g_i k=3(8) | g_{i+4} k=3(8)]
_GT_OFF = {"G0": 0, 1: (12, 22), 2: (32, 41), 3: (50, 58)}
_GT_W = 66


def _grid_tables(grid_row: np.ndarray, pre_scale: float = 1.0):
    """Build the (128, 66) constant table + per-level reciprocal immediates
    from one row of the (uniform) grid.

    pre_scale: the kernel feeds x' = pre_scale * x into the spline evaluator
    (layer 1 feeds raw per-channel sums, pre_scale = HWPIX). All grid knots
    are scaled by pre_scale so indicators/ratios are computed on x' directly.
    """
    g = np.asarray(grid_row, np.float64) * pre_scale
    assert g.shape == (12,)
    h = g[1] - g[0]
    tab = np.zeros((_GT_W,), np.float64)
    tab[0:12] = g
    rs = {}
    for k in (1, 2, 3):
        w = 11 - k
        aoff, coff = _GT_OFF[k]
        tab[aoff:aoff + w] = -g[:w]          # -g_i,      i = 0..10-k
        tab[coff:coff + w] = g[k + 1:12]     # g_{i+k+1}, i = 0..10-k
        rs[k] = float(np.float32(1.0 / (k * h)))
    full = np.tile(tab.astype(np.float32)[None, :], (128, 1))
    return np.ascontiguousarray(full), rs


def _host_prep(inputs):
    """Rearrange weights into the SBUF layouts the device program uses."""
    f32 = np.float32
    base_w1 = np.asarray(inputs["base_w1"], f32)      # (64, 512)
    spline_w1 = np.asarray(inputs["spline_w1"], f32)  # (64, 512, 8)
    scaler1 = np.asarray(inputs["scaler1"], f32)      # (64, 512)
    base_w2 = np.asarray(inputs["base_w2"], f32)      # (512, 64)
    spline_w2 = np.asarray(inputs["spline_w2"], f32)  # (512, 64, 8)
    scaler2 = np.asarray(inputs["scaler2"], f32)      # (512, 64)

    # Layer-1 base path consumes q = s*sigmoid(s/HWPIX) (= HWPIX*silu(mean)):
    # fold the 1/HWPIX into the base weights.
    # w1t[p, g*64+o] = base_w1[o, 128g+p] / HWPIX
    w1t = base_w1.reshape(HIDDEN, NG, 128).transpose(2, 1, 0).reshape(128, NG * HIDDEN)
    w1t = w1t / float(HWPIX)
    # sw1[p, (g*8+k)*64+o] = (spline_w1*scaler1)[o, 128g+p, k]
    sw1 = (spline_w1 * scaler1[:, :, None]).reshape(HIDDEN, NG, 128, KB)
    sw1 = sw1.transpose(2, 1, 3, 0).reshape(128, NG * KB * HIDDEN)
    # w2t[p, o] = base_w2[o, p]
    w2t = base_w2.T
    # sw2[p, k*512+o] = (spline_w2*scaler2)[o, p, k]
    sw2 = (spline_w2 * scaler2[:, :, None]).transpose(1, 2, 0).reshape(HIDDEN, KB * C)

    # Layer 1 evaluates splines on raw per-channel sums: fold the 1/HWPIX
    # mean into the knot tables.
    gt1, rs1 = _grid_tables(np.asarray(inputs["grid1"], f32)[0], pre_scale=float(HWPIX))
    gt2, rs2 = _grid_tables(np.asarray(inputs["grid2"], f32)[0])

    tensors = {
        "w1t": np.ascontiguousarray(w1t, f32),
        "sw1": np.ascontiguousarray(sw1, f32),
        "w2t": np.ascontiguousarray(w2t, f32),
        "sw2": np.ascontiguousarray(sw2, f32),
        "gt1": gt1,
        "gt2": gt2,
    }
    return tensors, rs1, rs2


def _emit_bsplines(nc, mybir, pool, gt_sb, x_ap, out_ap, p, rs):
    """Cubic B-spline bases of x (one value per partition) -> out_ap (p, 8).

    Cox-de-Boor on VectorE with per-basis-index grid constants from gt_sb
    and uniform-knot reciprocals rs (immediates).
    """
    f32 = mybir.dt.float32
    Alu = mybir.AluOpType
    ge = pool.tile([128, 12], f32, tag="ge", bufs=4)
    # ge[:, i] = (g_i <= x)
    nc.vector.tensor_scalar(
        out=ge[:p], in0=gt_sb[:p, 0:12], scalar1=x_ap, scalar2=None, op0=Alu.is_le
    )
    bprev = pool.tile([128, 11], f32, tag="b0", bufs=4)
    nc.vector.tensor_tensor(bprev[:p], ge[:p, 0:11], ge[:p, 1:12], Alu.subtract)
    for k in (1, 2, 3):
        w = 11 - k
        aoff, coff = _GT_OFF[k]
        a_t = pool.tile([128, 10], f32, tag="bsA", bufs=4)
        c_t = pool.tile([128, 10], f32, tag="bsC", bufs=4)
        # A = (x - g_i) / (k h);  C = (g_{i+k+1} - x) / (k h)
        nc.vector.tensor_scalar(
            out=a_t[:p, :w], in0=gt_sb[:p, aoff:aoff + w], scalar1=x_ap,
            scalar2=rs[k], op0=Alu.add, op1=Alu.mult,
        )
        nc.vector.tensor_scalar(
            out=c_t[:p, :w], in0=gt_sb[:p, coff:coff + w], scalar1=x_ap,
            scalar2=rs[k], op0=Alu.subtract, op1=Alu.mult,
        )
        if k < 3:
            bnext = pool.tile([128, 10], f32, tag="bn", bufs=4)
            outp = bnext[:p, :w]
        else:
            outp = out_ap
        nc.vector.tensor_tensor(c_t[:p, :w], c_t[:p, :w], bprev[:p, 1:w + 1], Alu.mult)
        nc.vector.tensor_tensor(outp, a_t[:p, :w], bprev[:p, 0:w], Alu.mult)
        nc.vector.tensor_tensor(outp, outp, c_t[:p, :w], Alu.add)
        if k < 3:
            bprev = bnext


def _build_nc(rs1, rs2):
    import concourse.bacc as bacc
    import concourse.bass as bass  # noqa: F401
    import concourse.mybir as mybir
    from concourse.tile import TileContext

    f32 = mybir.dt.float32
    bf16 = mybir.dt.bfloat16
    Alu = mybir.AluOpType
    Act = mybir.ActivationFunctionType

    # Bacc (not plain Bass): its compile() runs move_matmul_waits_to_ldweights
    # + generate_event_semaphores, which split multi-waits down to the 1-wait-
    # per-instruction TRN2 ISA limit that walrus enforces.
    nc = bacc.Bacc("TRN2", target_bir_lowering=False)
    x_d = nc.declare_dram_parameter("x", [NS, C, H, W], f32, isOutput=False)
    w1t_d = nc.declare_dram_parameter("w1t", [128, NG * HIDDEN], f32, isOutput=False)
    sw1_d = nc.declare_dram_parameter("sw1", [128, NG * KB * HIDDEN], f32, isOutput=False)
    w2t_d = nc.declare_dram_parameter("w2t", [HIDDEN, C], f32, isOutput=False)
    sw2_d = nc.declare_dram_parameter("sw2", [HIDDEN, KB * C], f32, isOutput=False)
    gt1_d = nc.declare_dram_parameter("gt1", [128, _GT_W], f32, isOutput=False)
    gt2_d = nc.declare_dram_parameter("gt2", [128, _GT_W], f32, isOutput=False)
    y_d = nc.declare_dram_parameter("y", [NS, C, H, W], f32, isOutput=True)

    with TileContext(nc) as tc:
        with (
            tc.tile_pool(name="consts", bufs=1) as cpool,
            tc.tile_pool(name="xstage", bufs=3) as stpool,
            tc.tile_pool(name="xdata", bufs=NS * NG) as xpool,
            tc.tile_pool(name="small", bufs=3) as spool,
            tc.tile_pool(name="bspl", bufs=1) as bpool,
            tc.tile_pool(name="psum", bufs=2, space="PSUM") as ppool,
        ):
            # Constants on the ACT HWDGE ring: off the x-load (SP) ring.
            w1t_sb = cpool.tile([128, NG * HIDDEN], f32)
            nc.scalar.dma_start(w1t_sb[:], w1t_d[:, :])
            sw1_sb = cpool.tile([128, NG * KB * HIDDEN], f32)
            nc.scalar.dma_start(sw1_sb[:], sw1_d[:, :])
            w2t_sb = cpool.tile([HIDDEN, C], f32)
            nc.scalar.dma_start(w2t_sb[:], w2t_d[:, :])
            sw2_sb = cpool.tile([HIDDEN, KB * C], f32)
            nc.scalar.dma_start(sw2_sb[:], sw2_d[:, :])
            gt1_sb = cpool.tile([128, _GT_W], f32)
            nc.scalar.dma_start(gt1_sb[:], gt1_d[:, :])
            gt2_sb = cpool.tile([128, _GT_W], f32)
            nc.scalar.dma_start(gt2_sb[:], gt2_d[:, :])

            # Pre-touch every const tile on VectorE: the DMA-completion wait
            # lands on these throwaway copies, so later DVE consumers (notably
            # TensorScalarPtr ops, whose ISA format has a single wait slot)
            # never need a DMA wait of their own.
            touch = cpool.tile([128, 8], f32)
            for i, ct in enumerate((w1t_sb, sw1_sb, gt1_sb, gt2_sb)):
                nc.vector.tensor_copy(touch[:, i:i + 1], ct[:, 0:1])
            for i, ct in enumerate((w2t_sb, sw2_sb)):
                nc.vector.tensor_copy(touch[:HIDDEN, 4 + i:5 + i], ct[:, 0:1])
            # Same for TensorE: the LDWEIGHTS sub-instruction also has a single
            # wait slot, so absorb each weight tile's DMA wait into a throwaway
            # 1-column matmul before the real accumulation chains.
            pt_ps = ppool.tile([1, 4], f32, tag="pt")
            for i, ct in enumerate((w1t_sb, sw1_sb)):
                nc.tensor.matmul(pt_ps[0:1, i:i + 1], ct[:, 0:1], ct[:, 0:1],
                                 start=True, stop=True)
            for i, ct in enumerate((w2t_sb, sw2_sb)):
                nc.tensor.matmul(pt_ps[0:1, 2 + i:3 + i], ct[:HIDDEN, 0:1],
                                 ct[:HIDDEN, 0:1], start=True, stop=True)

            # ---- ALL x loads up front on the SP HWDGE ring, f32 staging ----
            xst = [[None] * NG for _ in range(NS)]
            for n in range(NS):
                for g in range(NG):
                    xs = stpool.tile([128, HWPIX], f32, tag="xs")
                    src = x_d[n, 128 * g:128 * (g + 1)].rearrange("p h w -> p (h w)")
                    nc.sync.dma_start(xs[:], src)
                    xst[n][g] = xs

            xts = [[None] * NG for _ in range(NS)]
            sTs = [None] * NS

            def emit_convert_reduce(n):
                # ---- fused convert+reduce on ScalarE: f32 staging -> bf16
                # resident tile, accum_out = per-channel raw sum ----
                sT_n = spool.tile([128, NG], f32, tag="sT", bufs=4)
                sTs[n] = sT_n
                for g in range(NG):
                    xt = xpool.tile([128, HWPIX], bf16, tag="xt")
                    nc.scalar.activation(
                        xt[:], xst[n][g][:], Act.Copy,
                        accum_out=sTs[n][:, g:g + 1],
                    )
                    xts[n][g] = xt

            # Convert blocks are emitted one sample AHEAD of the KAN so that
            # on the in-order ScalarE stream sample n+1's converts (which
            # recycle the staging slots the later loads need) run before
            # sample n's sigmoid/gate chain, keeping the load ring paced by
            # conversion throughput rather than by gate latency.
            emit_convert_reduce(0)
            for n in range(NS):
                if n + 1 < NS:
                    emit_convert_reduce(n + 1)
                sT = sTs[n]

                # ---- KAN layer 1: s (512,) -> h1 (64,) ----
                # base path input q = s * sigmoid(s/HWPIX) (= HWPIX*silu(mean);
                # the 1/HWPIX lives in w1t). Splines read raw sums against
                # HWPIX-scaled knots.
                sg = spool.tile([128, NG], f32, tag="sg")
                nc.scalar.activation(sg[:], sT[:], Act.Sigmoid, scale=1.0 / HWPIX)
                q1 = spool.tile([128, NG], f32, tag="q1")
                nc.vector.tensor_tensor(q1[:], sg[:], sT[:], Alu.mult)
                bf = spool.tile([128, NG * KB], f32, tag="bf")
                for g in range(NG):
                    _emit_bsplines(
                        nc, mybir, bpool, gt1_sb, sT[:, g:g + 1],
                        bf[:, KB * g:KB * (g + 1)], 128, rs1,
                    )
                ps1 = ppool.tile([HIDDEN, 1], f32, tag="ps1")
                mms = []
                for g in range(NG):
                    mms.append((w1t_sb[:, HIDDEN * g:HIDDEN * (g + 1)], q1[:, g:g + 1]))
                for g in range(NG):
                    for k in range(KB):
                        col = HIDDEN * (KB * g + k)
                        mms.append((sw1_sb[:, col:col + HIDDEN], bf[:, KB * g + k:KB * g + k + 1]))
                for i, (lhsT, rhs) in enumerate(mms):
                    nc.tensor.matmul(
                        ps1[:], lhsT, rhs, start=(i == 0), stop=(i == len(mms) - 1)
                    )

                # ---- inter-layer SiLU (t1 = ps1*sigmoid(ps1)), layer 2 ----
                sg1 = spool.tile([HIDDEN, 1], f32, tag="sg1")
                nc.scalar.activation(sg1[:], ps1[:], Act.Sigmoid)
                t1 = spool.tile([HIDDEN, 1], f32, tag="t1")
                nc.vector.tensor_tensor(t1[:], sg1[:], ps1[:], Alu.mult)
                sg2 = spool.tile([HIDDEN, 1], f32, tag="sg2")
                nc.scalar.activation(sg2[:], t1[:], Act.Sigmoid)
                silu2 = spool.tile([HIDDEN, 1], f32, tag="silu2")
                nc.vector.tensor_tensor(silu2[:], sg2[:], t1[:], Alu.mult)
                b2f = spool.tile([HIDDEN, KB], f32, tag="b2f")
                _emit_bsplines(nc, mybir, bpool, gt2_sb, t1[:, 0:1], b2f[:], HIDDEN, rs2)

                ps2 = ppool.tile([128, NG], f32, tag="ps2")
                for og in range(NG):
                    mms2 = [(w2t_sb[:, 128 * og:128 * (og + 1)], silu2[:, 0:1])]
                    for k in range(KB):
                        col = C * k + 128 * og
                        mms2.append((sw2_sb[:, col:col + 128], b2f[:, k:k + 1]))
                    for i, (lhsT, rhs) in enumerate(mms2):
                        nc.tensor.matmul(
                            ps2[:, og:og + 1], lhsT, rhs,
                            start=(i == 0), stop=(i == len(mms2) - 1),
                        )

                gate = spool.tile([128, NG], f32, tag="gate")
                nc.scalar.activation(gate[:], ps2[:], Act.Sigmoid)

                # ---- scale resident bf16 tiles in place (VectorE), then
                # store on the SWDGE ring with a bf16->f32 cast ----
                for g in range(NG):
                    nc.vector.tensor_scalar(
                        out=xts[n][g][:], in0=xts[n][g][:], scalar1=gate[:, g:g + 1],
                        scalar2=None, op0=Alu.mult,
                    )
                    dst = y_d[n, 128 * g:128 * (g + 1)].rearrange("p h w -> p (h w)")
                    nc.gpsimd.dma_start(dst, xts[n][g][:])
    nc.compile()
    return nc


def _run(inputs, trace=False):
    from concourse.bass_utils import run_bass_kernel_spmd

    x = np.ascontiguousarray(np.asarray(inputs["x"], np.float32))
    assert x.shape == (B, C, H, W), x.shape
    tensors, rs1, rs2 = _host_prep(inputs)
    nc = _build_nc(rs1, rs2)
    in_maps = []
    for c in range(NCORES):
        m = {"x": np.ascontiguousarray(x[NS * c:NS * (c + 1)])}
        m.update(tensors)
        in_maps.append(m)
    res = run_bass_kernel_spmd(
        nc, in_maps, core_ids=list(range(NCORES)), trace=trace
    )
    out = np.concatenate([res.results[c]["y"] for c in range(NCORES)], axis=0)
    return out, res


def kernel(**inputs) -> np.ndarray:
    return _run(inputs)[0]


# revision 10
# speedup vs baseline: 1.3235x; 1.3235x over previous
"""KAN-SE (squeeze-excite with 2-layer KAN MLP) Trainium2 kernel.

Full-input contract: kernel(**inputs) takes the complete (32, 512, 64, 64)
batch plus KAN weights, shards the batch across 8 NeuronCores (4 samples
per core, data-parallel, weights replicated), and returns the full output.

Per-core device program (pure SPMD, no collectives), built so the load
and store streams overlap fully and share the ~358 GB/s per-core HBM
bandwidth:

  - ALL 16 x-tiles (4 samples x 4 channel-groups of (128, 4096) f32) are
    loaded up front on the SP HWDGE ring into a 3-deep f32 staging pool;
    the load DMAs are gated only by staging-slot recycling.
  - Each staged tile is converted f32->bf16 into the resident pool by a
    ScalarE Copy activation whose accum_out simultaneously produces the
    per-channel raw sum (the reduce rides the conversion). bf16 residency
    halves SBUF so all 4 samples stay on-chip.
  - Per-sample 2-layer KAN on the sums. The mean normalization is folded
    into the layer-1 knot tables / base weights; B-spline bases via
    Cox-de-Boor on VectorE; activations via Sigmoid only (silu(x) =
    x*sigmoid(x) with a DVE multiply) so the ScalarE table loads once.
    The einsums run as PAIRED PE matmuls to halve instruction overhead:
      layer 1: the 36 (128x64) weight blocks are packed pairwise into 18
        (128x128) stationary loads with a 2-column rhs; both pair halves
        accumulate into a (128, 2) PSUM and one DVE add extracts
        h[o] = ps[o, 0] + ps[64+o, 1].
      layer 2: spline blocks k=2j,2j+1 are stacked across partitions
        (contraction 64 -> 128), rhs columns [b_{2j}; b_{2j+1}] built by
        two strided DVE copies; 5 matmuls per output group instead of 9.
  - Gate scales applied in-place on the bf16 tiles, split 2 on ScalarE /
    2 on VectorE per sample; stores are bf16->f32 casting DMAs on the
    SWDGE (Pool) ring - its only traffic - so they drain in gate order
    concurrently with the remaining loads on the SP ring.
  - Weight/table constants load on the ACT HWDGE ring, off the load path.

HBM traffic is the 2x minimum (8 MiB in + 8 MiB out per sample per core);
the bf16 cast costs ~0.2% relative error on the output, well inside the
tolerance.
"""

import numpy as np

# ---- problem constants (hardcoded per contract; do not read spec/reference) ----
B, C, H, W = 32, 512, 64, 64
HIDDEN = 64            # max(16, 512 // 8)
KB = 8                 # GRID_SIZE + SPLINE_ORDER = 5 + 3
NCORES = 8
NS = B // NCORES       # samples per core = 4
NG = C // 128          # channel groups of 128 = 4
HWPIX = H * W          # 4096
NB1 = NG * KB + NG     # layer-1 weight blocks: 32 spline + 4 base = 36
NP1 = NB1 // 2         # 18 paired stationary loads

# gtab column layout: [G0(12) | -g_i for k=1(10) | g_{i+2} k=1(10)
#                      | -g_i k=2(9) | g_{i+3} k=2(9) | -g_i k=3(8) | g_{i+4} k=3(8)]
_GT_OFF = {"G0": 0, 1: (12, 22), 2: (32, 41), 3: (50, 58)}
_GT_W = 66


def _grid_tables(grid_row: np.ndarray, pre_scale: float = 1.0):
    """Build the (128, 66) constant table + per-level reciprocal immediates
    from one row of the (uniform) grid.

    pre_scale: the kernel feeds x' = pre_scale * x into the spline evaluator
    (layer 1 feeds raw per-channel sums, pre_scale = HWPIX). All grid knots
    are scaled by pre_scale so indicators/ratios are computed on x' directly.
    """
    g = np.asarray(grid_row, np.float64) * pre_scale
    assert g.shape == (12,)
    h = g[1] - g[0]
    tab = np.zeros((_GT_W,), np.float64)
    tab[0:12] = g
    rs = {}
    for k in (1, 2, 3):
        w = 11 - k
        aoff, coff = _GT_OFF[k]
        tab[aoff:aoff + w] = -g[:w]          # -g_i,      i = 0..10-k
        tab[coff:coff + w] = g[k + 1:12]     # g_{i+k+1}, i = 0..10-k
        rs[k] = float(np.float32(1.0 / (k * h)))
    full = np.tile(tab.astype(np.float32)[None, :], (128, 1))
    return np.ascontiguousarray(full), rs


def _host_prep(inputs):
    """Rearrange weights into the SBUF layouts the device program uses."""
    f32 = np.float32
    base_w1 = np.asarray(inputs["base_w1"], f32)      # (64, 512)
    spline_w1 = np.asarray(inputs["spline_w1"], f32)  # (64, 512, 8)
    scaler1 = np.asarray(inputs["scaler1"], f32)      # (64, 512)
    base_w2 = np.asarray(inputs["base_w2"], f32)      # (512, 64)
    spline_w2 = np.asarray(inputs["spline_w2"], f32)  # (512, 64, 8)
    scaler2 = np.asarray(inputs["scaler2"], f32)      # (512, 64)

    # Layer-1 blocks in rhs-column order: j<32 spline (g = j//8, k = j%8),
    # j>=32 base path for group g = j-32 (consumes q = s*sigmoid(s/HWPIX) =
    # HWPIX*silu(mean), so the base weights carry the 1/HWPIX).
    # sw1[p, (g*8+k)*64+o] = (spline_w1*scaler1)[o, 128g+p, k]
    sw1 = (spline_w1 * scaler1[:, :, None]).reshape(HIDDEN, NG, 128, KB)
    sw1 = sw1.transpose(2, 1, 3, 0).reshape(128, NG * KB * HIDDEN)
    # w1t[p, g*64+o] = base_w1[o, 128g+p] / HWPIX
    w1t = base_w1.reshape(HIDDEN, NG, 128).transpose(2, 1, 0).reshape(128, NG * HIDDEN)
    w1t = w1t / float(HWPIX)
    wpack1 = np.concatenate([sw1, w1t], axis=1)       # (128, 36*64)

    # Layer 2: base blocks w2t[p, o] = base_w2[o, p]; spline blocks stacked
    # pairwise across partitions: w2pack[(0:64|64:128), (og*4+j)*128+c] =
    # (spline_w2*scaler2)[128og+c, p, 2j(+1)]
    w2t = np.ascontiguousarray(base_w2.T)             # (64, 512)
    sw2 = (spline_w2 * scaler2[:, :, None]).transpose(1, 2, 0)  # (64, 8, 512)
    w2pack = np.zeros((128, NG * 4 * 128), f32)
    for og in range(NG):
        for j in range(4):
            blk = slice((og * 4 + j) * 128, (og * 4 + j + 1) * 128)
            w2pack[0:64, blk] = sw2[:, 2 * j, 128 * og:128 * (og + 1)]
            w2pack[64:128, blk] = sw2[:, 2 * j + 1, 128 * og:128 * (og + 1)]

    # Layer 1 evaluates splines on raw per-channel sums: fold the 1/HWPIX
    # mean into the knot tables.
    gt1, rs1 = _grid_tables(np.asarray(inputs["grid1"], f32)[0], pre_scale=float(HWPIX))
    gt2, rs2 = _grid_tables(np.asarray(inputs["grid2"], f32)[0])

    tensors = {
        "wpack1": np.ascontiguousarray(wpack1, f32),
        "w2pack": np.ascontiguousarray(w2pack, f32),
        "w2t": np.ascontiguousarray(w2t, f32),
        "gt1": gt1,
        "gt2": gt2,
    }
    return tensors, rs1, rs2


def _emit_bsplines(nc, mybir, pool, gt_sb, x_ap, out_ap, p, rs):
    """Cubic B-spline bases of x (one value per partition) -> out_ap (p, 8).

    Cox-de-Boor on VectorE with per-basis-index grid constants from gt_sb
    and uniform-knot reciprocals rs (immediates).
    """
    f32 = mybir.dt.float32
    Alu = mybir.AluOpType
    ge = pool.tile([128, 12], f32, tag="ge", bufs=4)
    # ge[:, i] = (g_i <= x)
    nc.vector.tensor_scalar(
        out=ge[:p], in0=gt_sb[:p, 0:12], scalar1=x_ap, scalar2=None, op0=Alu.is_le
    )
    bprev = pool.tile([128, 11], f32, tag="b0", bufs=4)
    nc.vector.tensor_tensor(bprev[:p], ge[:p, 0:11], ge[:p, 1:12], Alu.subtract)
    for k in (1, 2, 3):
        w = 11 - k
        aoff, coff = _GT_OFF[k]
        a_t = pool.tile([128, 10], f32, tag="bsA", bufs=4)
        c_t = pool.tile([128, 10], f32, tag="bsC", bufs=4)
        # A = (x - g_i) / (k h);  C = (g_{i+k+1} - x) / (k h)
        nc.vector.tensor_scalar(
            out=a_t[:p, :w], in0=gt_sb[:p, aoff:aoff + w], scalar1=x_ap,
            scalar2=rs[k], op0=Alu.add, op1=Alu.mult,
        )
        nc.vector.tensor_scalar(
            out=c_t[:p, :w], in0=gt_sb[:p, coff:coff + w], scalar1=x_ap,
            scalar2=rs[k], op0=Alu.subtract, op1=Alu.mult,
        )
        if k < 3:
            bnext = pool.tile([128, 10], f32, tag="bn", bufs=4)
            outp = bnext[:p, :w]
        else:
            outp = out_ap
        nc.vector.tensor_tensor(c_t[:p, :w], c_t[:p, :w], bprev[:p, 1:w + 1], Alu.mult)
        nc.vector.tensor_tensor(outp, a_t[:p, :w], bprev[:p, 0:w], Alu.mult)
        nc.vector.tensor_tensor(outp, outp, c_t[:p, :w], Alu.add)
        if k < 3:
            bprev = bnext


def _build_nc(rs1, rs2):
    import concourse.bacc as bacc
    import concourse.bass as bass  # noqa: F401
    import concourse.mybir as mybir
    from concourse.tile import TileContext

    f32 = mybir.dt.float32
    bf16 = mybir.dt.bfloat16
    Alu = mybir.AluOpType
    Act = mybir.ActivationFunctionType

    # Bacc (not plain Bass): its compile() runs move_matmul_waits_to_ldweights
    # + generate_event_semaphores, which split multi-waits down to the 1-wait-
    # per-instruction TRN2 ISA limit that walrus enforces.
    nc = bacc.Bacc("TRN2", target_bir_lowering=False)
    x_d = nc.declare_dram_parameter("x", [NS, C, H, W], f32, isOutput=False)
    wp1_d = nc.declare_dram_parameter("wpack1", [128, NB1 * HIDDEN], f32, isOutput=False)
    wp2_d = nc.declare_dram_parameter("w2pack", [128, NG * 4 * 128], f32, isOutput=False)
    w2t_d = nc.declare_dram_parameter("w2t", [HIDDEN, C], f32, isOutput=False)
    gt1_d = nc.declare_dram_parameter("gt1", [128, _GT_W], f32, isOutput=False)
    gt2_d = nc.declare_dram_parameter("gt2", [128, _GT_W], f32, isOutput=False)
    y_d = nc.declare_dram_parameter("y", [NS, C, H, W], f32, isOutput=True)

    with TileContext(nc) as tc:
        with (
            tc.tile_pool(name="consts", bufs=1) as cpool,
            tc.tile_pool(name="xstage", bufs=3) as stpool,
            tc.tile_pool(name="xdata", bufs=NS * NG) as xpool,
            tc.tile_pool(name="small", bufs=3) as spool,
            tc.tile_pool(name="bspl", bufs=1) as bpool,
            tc.tile_pool(name="psum", bufs=2, space="PSUM") as ppool,
        ):
            # Constants on the ACT HWDGE ring: off the x-load (SP) ring.
            wp1_sb = cpool.tile([128, NB1 * HIDDEN], f32)
            nc.scalar.dma_start(wp1_sb[:], wp1_d[:, :])
            wp2_sb = cpool.tile([128, NG * 4 * 128], f32)
            nc.scalar.dma_start(wp2_sb[:], wp2_d[:, :])
            w2t_sb = cpool.tile([HIDDEN, C], f32)
            nc.scalar.dma_start(w2t_sb[:], w2t_d[:, :])
            gt1_sb = cpool.tile([128, _GT_W], f32)
            nc.scalar.dma_start(gt1_sb[:], gt1_d[:, :])
            gt2_sb = cpool.tile([128, _GT_W], f32)
            nc.scalar.dma_start(gt2_sb[:], gt2_d[:, :])

            # Pre-touch every const tile on VectorE: the DMA-completion wait
            # lands on these throwaway copies, so later DVE consumers (notably
            # TensorScalarPtr ops, whose ISA format has a single wait slot)
            # never need a DMA wait of their own.
            touch = cpool.tile([128, 8], f32)
            for i, ct in enumerate((wp1_sb, wp2_sb, gt1_sb, gt2_sb)):
                nc.vector.tensor_copy(touch[:, i:i + 1], ct[:, 0:1])
            nc.vector.tensor_copy(touch[:HIDDEN, 4:5], w2t_sb[:, 0:1])
            # Same for TensorE: the LDWEIGHTS sub-instruction also has a single
            # wait slot, so absorb each weight tile's DMA wait into a throwaway
            # 1-column matmul before the real accumulation chains.
            pt_ps = ppool.tile([1, 4], f32, tag="pt")
            for i, ct in enumerate((wp1_sb, wp2_sb)):
                nc.tensor.matmul(pt_ps[0:1, i:i + 1], ct[:, 0:1], ct[:, 0:1],
                                 start=True, stop=True)
            nc.tensor.matmul(pt_ps[0:1, 2:3], w2t_sb[:HIDDEN, 0:1],
                             w2t_sb[:HIDDEN, 0:1], start=True, stop=True)

            # ---- ALL x loads up front on the SP HWDGE ring, f32 staging ----
            xst = [[None] * NG for _ in range(NS)]
            for n in range(NS):
                for g in range(NG):
                    xs = stpool.tile([128, HWPIX], f32, tag="xs")
                    src = x_d[n, 128 * g:128 * (g + 1)].rearrange("p h w -> p (h w)")
                    nc.sync.dma_start(xs[:], src)
                    xst[n][g] = xs

            xts = [[None] * NG for _ in range(NS)]
            for n in range(NS):
                # ---- fused convert+reduce on ScalarE: f32 staging -> bf16
                # resident tile, accum_out = per-channel raw sum ----
                sT = spool.tile([128, NG], f32, tag="sT")
                for g in range(NG):
                    xt = xpool.tile([128, HWPIX], bf16, tag="xt")
                    nc.scalar.activation(
                        xt[:], xst[n][g][:], Act.Copy,
                        accum_out=sT[:, g:g + 1],
                    )
                    xts[n][g] = xt

                # ---- KAN layer 1: s (512,) -> h1 (64,) via 18 paired
                # matmuls; bfq holds the 36 rhs betas in block order ----
                bfq = spool.tile([128, NB1], f32, tag="bfq")
                sg = spool.tile([128, NG], f32, tag="sg")
                nc.scalar.activation(sg[:], sT[:], Act.Sigmoid, scale=1.0 / HWPIX)
                # base-path betas: q = s * sigmoid(s/HWPIX)  (cols 32..35)
                nc.vector.tensor_tensor(bfq[:, NG * KB:NB1], sg[:], sT[:], Alu.mult)
                for g in range(NG):
                    _emit_bsplines(
                        nc, mybir, bpool, gt1_sb, sT[:, g:g + 1],
                        bfq[:, KB * g:KB * (g + 1)], 128, rs1,
                    )
                psA = ppool.tile([128, 2], f32, tag="ps1")
                for j in range(NP1):
                    nc.tensor.matmul(
                        psA[:, 0:2], wp1_sb[:, 128 * j:128 * (j + 1)],
                        bfq[:, 2 * j:2 * j + 2],
                        start=(j == 0), stop=(j == NP1 - 1),
                    )
                hv = spool.tile([HIDDEN, 1], f32, tag="hv")
                # DVE may read only one PSUM operand per instruction
                nc.vector.tensor_copy(hv[:], psA[HIDDEN:128, 1:2])
                nc.vector.tensor_tensor(hv[:], hv[:], psA[0:HIDDEN, 0:1], Alu.add)

                # ---- inter-layer SiLU (t1 = h*sigmoid(h)), layer 2 ----
                sg1 = spool.tile([HIDDEN, 1], f32, tag="sg1")
                nc.scalar.activation(sg1[:], hv[:], Act.Sigmoid)
                t1 = spool.tile([HIDDEN, 1], f32, tag="t1")
                nc.vector.tensor_tensor(t1[:], sg1[:], hv[:], Alu.mult)
                sg2 = spool.tile([HIDDEN, 1], f32, tag="sg2")
                nc.scalar.activation(sg2[:], t1[:], Act.Sigmoid)
                silu2 = spool.tile([HIDDEN, 1], f32, tag="silu2")
                nc.vector.tensor_tensor(silu2[:], sg2[:], t1[:], Alu.mult)
                b2f = spool.tile([HIDDEN, KB], f32, tag="b2f")
                _emit_bsplines(nc, mybir, bpool, gt2_sb, t1[:, 0:1], b2f[:], HIDDEN, rs2)

                # stacked rhs: stk[0:64, j] = b2f[:, 2j], stk[64:128, j] = b2f[:, 2j+1]
                stk = spool.tile([128, 4], f32, tag="stk")
                nc.vector.tensor_copy(stk[0:HIDDEN, 0:4], b2f[:, 0:KB:2])
                nc.vector.tensor_copy(stk[HIDDEN:128, 0:4], b2f[:, 1:KB:2])

                ps2 = ppool.tile([128, NG], f32, tag="ps2")
                for og in range(NG):
                    nc.tensor.matmul(
                        ps2[:, og:og + 1], w2t_sb[:, 128 * og:128 * (og + 1)],
                        silu2[:, 0:1], start=True, stop=False,
                    )
                    for j in range(4):
                        blk = slice((og * 4 + j) * 128, (og * 4 + j + 1) * 128)
                        nc.tensor.matmul(
                            ps2[:, og:og + 1], wp2_sb[:, blk], stk[:, j:j + 1],
                            start=False, stop=(j == 3),
                        )

                gate = spool.tile([128, NG], f32, tag="gate")
                nc.scalar.activation(gate[:], ps2[:], Act.Sigmoid)

                # ---- scale resident bf16 tiles in place (2 on ScalarE, 2 on
                # VectorE), then store on the SWDGE ring (bf16->f32 cast) ----
                for g in range(NG):
                    if g < 2:
                        nc.scalar.mul(xts[n][g][:], xts[n][g][:], gate[:, g:g + 1])
                    else:
                        nc.vector.tensor_scalar(
                            out=xts[n][g][:], in0=xts[n][g][:],
                            scalar1=gate[:, g:g + 1], scalar2=None, op0=Alu.mult,
                        )
                    dst = y_d[n, 128 * g:128 * (g + 1)].rearrange("p h w -> p (h w)")
                    nc.gpsimd.dma_start(dst, xts[n][g][:])
    nc.compile()
    return nc


def _run(inputs, trace=False):
    from concourse.bass_utils import run_bass_kernel_spmd

    x = np.ascontiguousarray(np.asarray(inputs["x"], np.float32))
    assert x.shape == (B, C, H, W), x.shape
    tensors, rs1, rs2 = _host_prep(inputs)
    nc = _build_nc(rs1, rs2)
    in_maps = []
    for c in range(NCORES):
        m = {"x": np.ascontiguousarray(x[NS * c:NS * (c + 1)])}
        m.update(tensors)
        in_maps.append(m)
    res = run_bass_kernel_spmd(
        nc, in_maps, core_ids=list(range(NCORES)), trace=trace
    )
    out = np.concatenate([res.results[c]["y"] for c in range(NCORES)], axis=0)
    return out, res


def kernel(**inputs) -> np.ndarray:
    return _run(inputs)[0]
